# revision 16
# baseline (speedup 1.0000x reference)
"""Trainium2 Bass kernel for a GNN message-passing layer (BoundaryConvLayer).

Computation (reference, per node i over D=128 channels):
    rate  = softplus(x @ W_rate) + EPS
    gamma = x @ W_rob + b_rob
    h     = x @ W_fc + b_fc
    agg   = segment_sum(h[row] + h[col], row)
    y     = LayerNorm((rate*agg + gamma) / (1 + rate*deg + EPS)) * ln_gamma + ln_beta

Distribution: nodes sharded across 8 cores by contiguous row blocks; edges
partitioned by destination row so the segment sum is local.  Each core GEMMs
only the x rows its edges actually reference (host-side dedup, ~63k of 100k)
into a per-core DRAM gather table g = x_src @ W_fc; per-edge g[col] rows are
fetched with batched DMAGatherAnt.

Key identities:
    agg[i] = cnt[i]*g[i] + sum_{e:row=i} g[col_e] + 2*cnt[i]*b_fc
  where cnt = in-edge count (h = g + b_fc).  The neighbor sum is a one-hot
  "selection matrix" matmul on the PE over gathered edge rows; the self term
  cnt[i]*g[i] is one matmul with host-prescaled lhsT (cnt*x); both bias
  rank-1 terms are K<=4 matmuls issued once per 4-tile eltwise group.

Gather layout: dma_gather indices are int16, so the g table is split into
NCHK=2 chunks of CPAD rows (< 32768); per (tile, chunk) the edges fill Cq
128-slot groups (Cq = global max), zero-row pads in unused slots.  One dense
dma_gather per (chunk, 7-tile group).

LayerNorm: mean/var via bn_stats/bn_aggr on DVE, 1/den and rstd via
exp(-ln(.)) on the ACT engine (one activation table), final (y-mean)*rstd
as a single ACT Identity with per-partition scale/bias APs.
"""

import numpy as np
import ml_dtypes
from contextlib import ExitStack
from dataclasses import dataclass

import concourse.bass as bass
import concourse.tile as tile
from concourse import bacc, mybir
from concourse.bass_utils import run_bass_kernel_spmd

# The stock ACT-table chooser greedily picks the first set containing each
# function, which can alternate between sets and reload the table (~1.3us
# each).  Restrict it to the one set that contains all funcs we use
# (Exp, Ln, Copy, Identity, Square) so a single load suffices.
_ACT_KEEP = "natural_log_exp_and_others"
if not getattr(bacc, "_act_tables_patched", False):
    _orig_get_tables = bacc.get_activation_tables

    def _patched_get_tables(arch):
        t = _orig_get_tables(arch)
        if _ACT_KEEP in t:
            t = {k: (v if k == _ACT_KEEP else set()) for k, v in t.items()}
        return t

    bacc.get_activation_tables = _patched_get_tables
    bacc._act_tables_patched = True

BF16 = ml_dtypes.bfloat16
EPS = 1e-4
LN_EPS = 1e-5
P = 128
D = 128


@dataclass
class Cfg:
    N: int            # total nodes
    E: int            # total edges
    NC: int           # cores
    NCHK: int = 2     # gather table chunks (int16 range)
    CPAD: int = 32000  # padded chunk rows (128-aligned, <= 32767)
    Cq: int = 0       # 128-slot groups per (tile, chunk); set by prep
    ln_trivial: bool = False

    @property
    def NLOC(self):
        return self.N // self.NC

    @property
    def T(self):
        return (self.NLOC + P - 1) // P

    @property
    def TLP(self):
        return self.T * P

    @property
    def RPCC(self):   # real rows per chunk (>=64 zero pad rows at chunk end)
        return self.CPAD - 64

    @property
    def G(self):      # tiles per gather group
        for g in (7, 14, 4, 2, 1):
            if self.T % g == 0:
                return g
        return 1

    @property
    def B(self):      # tiles per eltwise group
        return 4


def prep(x, edge_index, degree, W_fc, b_fc, W_rate, W_rob, b_rob, ln_gamma, ln_beta,
         cfg: Cfg):
    """Host-side preprocessing: shard + build per-core gather/selection tables."""
    N, NC, NCHK, CPAD = cfg.N, cfg.NC, cfg.NCHK, cfg.CPAD
    NLOC, T, TLP, RPCC = cfg.NLOC, cfg.T, cfg.TLP, cfg.RPCC
    B = cfg.B
    NG4 = (T + B - 1) // B

    x = np.asarray(x, np.float32)
    edge_index = np.asarray(edge_index, np.int64)
    degree = np.asarray(degree)
    row, col = edge_index[0], edge_index[1]

    w_fc = np.ascontiguousarray(W_fc, dtype=np.float32).astype(BF16)
    w_rt = np.ascontiguousarray(W_rate, dtype=np.float32).astype(BF16)
    w_rb = np.ascontiguousarray(W_rob, dtype=np.float32).astype(BF16)

    bfc4 = np.zeros((B, B * D), BF16)       # block-diag b_fc (rhs of K=4 bias)
    for j in range(B):
        bfc4[j, j * D:(j + 1) * D] = np.asarray(b_fc, np.float32).astype(BF16)
    brobt = np.tile(np.asarray(b_rob, np.float32).astype(BF16)[None, :], (1, B))
    onesr = np.ones((1, P), BF16)

    cfg.ln_trivial = bool(np.all(np.asarray(ln_gamma) == 1.0)
                          and np.all(np.asarray(ln_beta) == 0.0))
    lnab = np.zeros((P, 2 * D), np.float32)
    lnab[:, :D] = np.asarray(ln_gamma, np.float32)[None, :]
    lnab[:, D:] = np.asarray(ln_beta, np.float32)[None, :]

    core_of = row // NLOC

    # pass 1: per-core dedup + per-(tile,chunk) counts fix the global Cq
    percore = []
    maxslots = 0
    for r in range(NC):
        m = core_of == r
        rl = (row[m] - r * NLOC).astype(np.int64)
        ce = col[m]
        srcs, inv = np.unique(ce, return_inverse=True)
        S = len(srcs)
        assert S <= NCHK * RPCC, (S, NCHK * RPCC)
        cq = inv // RPCC
        crow = inv - cq * RPCC
        cnt = np.bincount(rl, minlength=TLP)
        tq = (rl // P) * NCHK + cq
        cnt_tq = np.bincount(tq, minlength=T * NCHK).reshape(T, NCHK)
        maxslots = max(maxslots, int(cnt_tq.max()))
        percore.append((rl, cq, crow, cnt, cnt_tq, srcs))
    Cq = max(1, -(-maxslots // P))
    cfg.Cq = Cq
    G = cfg.G
    NG = T // G
    IPG = G * Cq * P           # idxs per (chunk, group) instruction

    in_maps = []
    for r in range(NC):
        rl, cq, crow, cnt, cnt_tq, srcs = percore[r]
        # order edges by (tile, chunk, src-row): dense grids + HBM locality
        order = np.lexsort((crow, cq, rl // P))
        rl_s, cq_s, crow_s = rl[order], cq[order], crow[order]
        t_s = rl_s // P
        tq_s = t_s * NCHK + cq_s
        run_start = np.zeros(T * NCHK + 1, np.int64)
        np.cumsum(cnt_tq.reshape(-1), out=run_start[1:])
        pos = np.arange(len(rl_s)) - run_start[tq_s]
        tl_s = t_s % G
        gg_s = t_s // G
        ipos = tl_s * (Cq * P) + pos
        idx16 = np.full((NCHK, NG, IPG), CPAD - 1, np.int16)  # pad -> zero row
        idx16[cq_s, gg_s, ipos] = crow_s.astype(np.int16)
        # wrap each stream: idx i -> [i%16, i//16], replicate to 128 partitions
        idxw = idx16.reshape(NCHK, NG, IPG // 16, 16).transpose(0, 1, 3, 2)
        idxw = np.ascontiguousarray(idxw)
        idxw = np.tile(idxw, (1, 1, 8, 1))           # [NCHK, NG, 128, IPG//16]
        idx_sb = np.ascontiguousarray(
            idxw.transpose(2, 0, 1, 3)).reshape(P, NCHK * NG * (IPG // 16))

        # rowsr: rebased row (node % 128) per slot, -1 for pads
        rowsr = np.full((P, T * NCHK * Cq), -1.0, BF16)
        slot_col = t_s * (NCHK * Cq) + cq_s * Cq + pos // P
        rowsr[pos % P, slot_col] = (rl_s % P).astype(BF16)

        iotab = np.ascontiguousarray(
            np.tile(np.arange(P, dtype=BF16)[None, :], (P, NCHK * Cq)))

        # dense source x, chunk-major, transposed; zero rows at chunk tails
        xs = np.zeros((NCHK * CPAD, D), np.float32)
        for q in range(NCHK):
            seg = srcs[q * RPCC:(q + 1) * RPCC]
            if len(seg):
                xs[q * CPAD:q * CPAD + len(seg)] = x[seg]
        XS = np.ascontiguousarray(xs.T.astype(BF16))

        xl = x[r * NLOC:(r + 1) * NLOC]
        xloc = np.zeros((P, TLP), BF16)
        xloc[:, :NLOC] = xl.T.astype(BF16)
        xls = np.zeros((P, TLP), BF16)
        xls[:, :NLOC] = (xl * cnt[:NLOC, None]).T.astype(BF16)

        cnt4 = np.zeros((B, NG4 * P), BF16)   # lhsT of the K=4 cnt*b_fc bias
        c2 = (2.0 * cnt).astype(np.float32)
        for g in range(NG4):
            for k in range(B):
                t = g * B + k
                if t < T:
                    cnt4[k, g * P:(g + 1) * P] = c2[t * P:(t + 1) * P].astype(BF16)

        degl = np.zeros(TLP, np.float32)
        degl[:NLOC] = degree[r * NLOC:(r + 1) * NLOC].astype(np.float32)
        degf = np.ascontiguousarray(degl.reshape(T, P).T)

        in_maps.append({
            "XS": XS, "xloc": xloc, "xls": xls,
            "Wfc": w_fc, "Wrt": w_rt, "Wrb": w_rb,
            "bfc4": bfc4, "brobt": brobt, "onesr": onesr, "lnab": lnab,
            "iotab": iotab, "rowsr": rowsr, "idxs": idx_sb,
            "cnt4": cnt4, "degf": degf,
        })
    return in_maps


def build(cfg: Cfg):
    """Build the SPMD Bass program (identical on every core)."""
    NC, T, TLP = cfg.NC, cfg.T, cfg.TLP
    NCHK, Cq, CPAD = cfg.NCHK, cfg.Cq, cfg.CPAD
    G, B = cfg.G, cfg.B
    NG = T // G
    NG4 = (T + B - 1) // B
    IPG = G * Cq * P
    SELW = NCHK * Cq * P       # sel width per tile
    bf = mybir.dt.bfloat16
    f32 = mybir.dt.float32
    i16 = mybir.dt.int16
    AF = mybir.ActivationFunctionType

    nc = bacc.Bacc("TRN2", target_bir_lowering=False, debug=False, num_devices=NC,
                   num_swdge_queues=4)
    for cval in (LN_EPS, 1.0 + EPS):
        cs = nc.alloc_sbuf_tensor(f"const-float32-{cval}", [P, 1], f32)
        nc.gpsimd.memset(cs.ap(), cval)
        nc.const_aps.aps[(f32, cval)] = cs.ap()
    nc.all_engine_barrier()

    d_XS = nc.dram_tensor("XS", [P, NCHK * CPAD], bf, kind="ExternalInput").ap()
    d_xloc = nc.dram_tensor("xloc", [P, TLP], bf, kind="ExternalInput").ap()
    d_xls = nc.dram_tensor("xls", [P, TLP], bf, kind="ExternalInput").ap()
    d_wfc = nc.dram_tensor("Wfc", [P, D], bf, kind="ExternalInput").ap()
    d_wrt = nc.dram_tensor("Wrt", [P, D], bf, kind="ExternalInput").ap()
    d_wrb = nc.dram_tensor("Wrb", [P, D], bf, kind="ExternalInput").ap()
    d_bfc4 = nc.dram_tensor("bfc4", [B, B * D], bf, kind="ExternalInput").ap()
    d_brobt = nc.dram_tensor("brobt", [1, B * D], bf, kind="ExternalInput").ap()
    d_ones = nc.dram_tensor("onesr", [1, P], bf, kind="ExternalInput").ap()
    d_lnab = nc.dram_tensor("lnab", [P, 2 * D], f32, kind="ExternalInput").ap()
    d_iota = nc.dram_tensor("iotab", [P, SELW], bf, kind="ExternalInput").ap()
    d_rowsr = nc.dram_tensor("rowsr", [P, T * NCHK * Cq], bf,
                             kind="ExternalInput").ap()
    d_idxs = nc.dram_tensor("idxs", [P, NCHK * NG * (IPG // 16)], i16,
                            kind="ExternalInput").ap()
    d_cnt4 = nc.dram_tensor("cnt4", [B, NG4 * P], bf, kind="ExternalInput").ap()
    d_degf = nc.dram_tensor("degf", [P, T], f32, kind="ExternalInput").ap()
    # one g-table tensor per chunk so chunk-q gathers depend only on chunk-q
    # phase-1 writes
    d_gq = [nc.dram_tensor(f"gtab{q}", [CPAD, D], bf, kind="Internal").ap()
            for q in range(NCHK)]
    d_y = nc.dram_tensor("y", [TLP, D], f32, kind="ExternalOutput").ap()

    with tile.TileContext(nc) as tc, ExitStack() as ctx:
        from concourse import library_config
        nc.gpsimd.load_library(library_config.mlp)
        consts = ctx.enter_context(tc.tile_pool(name="consts", bufs=1))
        wfc = consts.tile([P, D], bf)
        nc.sync.dma_start(wfc[:], d_wfc[:])
        # phase-3-only consts: allocate now, DMA after phase-1 issue so the
        # loads overlap the table GEMM instead of delaying it
        wrt = consts.tile([P, D], bf)
        wrb = consts.tile([P, D], bf)
        iota = consts.tile([P, SELW], bf)
        rowsr = consts.tile([P, T * NCHK * Cq], bf)
        idxs = consts.tile([P, NCHK * NG * (IPG // 16)], i16)
        cnt4 = consts.tile([B, NG4 * P], bf)
        bfc4 = consts.tile([B, B * D], bf)
        brobt = consts.tile([1, B * D], bf)
        onesr = consts.tile([1, P], bf)
        degf = consts.tile([P, T], f32)
        xloc = consts.tile([P, TLP], bf)
        xls = consts.tile([P, TLP], bf)
        lnab = consts.tile([P, 2 * D], f32) if not cfg.ln_trivial else None

        def load_phase3_consts():
            nc.sync.dma_start(wrt[:], d_wrt[:])
            nc.sync.dma_start(wrb[:], d_wrb[:])
            nc.sync.dma_start(iota[:], d_iota[:])
            nc.sync.dma_start(rowsr[:], d_rowsr[:])
            nc.sync.dma_start(idxs[:], d_idxs[:])
            nc.sync.dma_start(cnt4[:], d_cnt4[:])
            nc.sync.dma_start(bfc4[:], d_bfc4[:])
            nc.sync.dma_start(brobt[:], d_brobt[:])
            nc.sync.dma_start(onesr[:], d_ones[:])
            nc.sync.dma_start(degf[:], d_degf[:])
            nc.sync.dma_start(xloc[:], d_xloc[:])
            nc.sync.dma_start(xls[:], d_xls[:])
            if lnab is not None:
                nc.sync.dma_start(lnab[:], d_lnab[:])

        # phase-3 psum pools first: disjoint banks from phase-1's pool
        apsp = ctx.enter_context(tc.tile_pool(name="apsp", bufs=2, space="PSUM"))
        ratp = ctx.enter_context(tc.tile_pool(name="ratp", bufs=2, space="PSUM"))
        gamp = ctx.enter_context(tc.tile_pool(name="gamp", bufs=2, space="PSUM"))
        msgp = ctx.enter_context(tc.tile_pool(name="msgp", bufs=3))
        selp = ctx.enter_context(tc.tile_pool(name="selp", bufs=8))
        eltp = ctx.enter_context(tc.tile_pool(name="eltp", bufs=2))
        smallp = ctx.enter_context(tc.tile_pool(name="smallp", bufs=2))

        # ---------------- phase 1: g = x_src @ W_fc (dedup'd) ----------------
        CHUNK = 4096
        GRP = 512
        with tc.tile_pool(name="p1x", bufs=2) as p1x, \
             tc.tile_pool(name="p1ps", bufs=2, space="PSUM") as p1ps, \
             tc.tile_pool(name="p1st", bufs=6) as p1st:
            ng = 0
            # interleave the two chunks so both gather tables complete
            # (almost) simultaneously -- chunk-1 gathers are on the critical
            # path and must not wait for a sequential full phase 1
            for c0 in range(0, CPAD, CHUNK):
                for q in range(NCHK):
                    cw = min(CHUNK, CPAD - c0)
                    xc = p1x.tile([P, CHUNK], bf, tag="xc", name="xc")
                    nc.sync.dma_start(xc[:, :cw],
                                      d_XS[:, q * CPAD + c0:q * CPAD + c0 + cw])
                    for g0 in range(0, cw, GRP):
                        gw = min(GRP, cw - g0)
                        gps = p1ps.tile([P, GRP], f32, space="PSUM", tag="gps",
                                        name="gps")
                        for j in range(0, gw, P):
                            nc.tensor.matmul(
                                out=gps[:, j:j + P],
                                lhsT=xc[:, g0 + j:g0 + j + P],
                                rhs=wfc[:],
                                start=True, stop=True,
                            )
                        gst = p1st.tile([P, GRP], bf, tag="gst", name="gst")
                        # alternate the psum->bf16 conversion between the two
                        # free elementwise engines so neither paces phase 1
                        if ng & 1:
                            nc.scalar.copy(gst[:, :gw], gps[:, :gw])
                        else:
                            nc.vector.tensor_copy(gst[:, :gw], gps[:, :gw])
                        ng += 1
                        dst = d_gq[q][c0 + g0:c0 + g0 + gw, :].rearrange(
                            "(t p) d -> p t d", p=P)
                        nc.sync.dma_start(dst, gst[:, :gw].rearrange(
                            "p (t d) -> p t d", d=D))

        load_phase3_consts()

        # ---------------- phase 3: message passing + elementwise -------------
        bt = None

        def eltwise(bt, t0, nb):
            aps4, rps4, gps4, rate4 = bt
            a3 = aps4[:, :nb * D].rearrange("p (b d) -> p b d", d=D)
            g3 = gps4[:, :nb * D].rearrange("p (b d) -> p b d", d=D)
            r3 = rate4[:, :nb, :]
            num = eltp.tile([P, B, D], f32, tag="num", name="num")[:, :nb, :]
            den = eltp.tile([P, B, D], f32, tag="den", name="den")[:, :nb, :]
            y0 = eltp.tile([P, B, D], f32, tag="y0", name="y0")
            yf = eltp.tile([P, B, D], f32, tag="yf", name="yf")
            st6 = smallp.tile([P, B, 6], f32, tag="st6", name="st6")
            st2 = smallp.tile([P, B, 2], f32, tag="st2", name="st2")
            sm = smallp.tile([P, 4 * B], f32, tag="sm", name="sm")
            lnv = sm[:, 0:nb]
            rstd = sm[:, B:B + nb]
            q4 = sm[:, 2 * B:2 * B + nb]
            y03 = y0[:, :nb, :]

            # num = (softplus + EPS) * agg + gamma
            nc.vector.scalar_tensor_tensor(
                out=num, in0=r3, scalar=EPS, in1=a3,
                op0=mybir.AluOpType.add, op1=mybir.AluOpType.mult)
            nc.vector.tensor_add(out=num, in0=num, in1=g3)
            # den = (softplus + EPS) * deg;  1/(den + 1 + EPS) = exp(-ln(.))
            degb = degf[:, t0:t0 + nb][:, :, None].to_broadcast([P, nb, D])
            nc.vector.scalar_tensor_tensor(
                out=den, in0=r3, scalar=EPS, in1=degb,
                op0=mybir.AluOpType.add, op1=mybir.AluOpType.mult)
            nc.scalar.activation(out=den, in_=den, func=AF.Ln, bias=1.0 + EPS)
            nc.scalar.activation(out=den, in_=den, func=AF.Exp, scale=-1.0)
            nc.vector.tensor_mul(out=y03, in0=num, in1=den)
            # LayerNorm stats: bn_stats/bn_aggr give per-tile (mean, var)
            for j in range(nb):
                nc.vector.bn_stats(out=st6[:, j, :], in_=y0[:, j, :])
                nc.vector.bn_aggr(out=st2[:, j, :], in_=st6[:, j, :])
            nc.scalar.activation(out=lnv, in_=st2[:, :nb, 1], func=AF.Ln,
                                 bias=LN_EPS)
            nc.scalar.activation(out=rstd, in_=lnv, func=AF.Exp, scale=-0.5)
            # q4 = -mean * rstd;  yf = y0 * rstd + q4  (per-partition ACT affine)
            nc.vector.scalar_tensor_tensor(
                out=q4, in0=st2[:, :nb, 0], scalar=-1.0, in1=rstd,
                op0=mybir.AluOpType.mult, op1=mybir.AluOpType.mult)
            for j in range(nb):
                nc.scalar.activation(out=yf[:, j, :], in_=y0[:, j, :],
                                     func=AF.Identity,
                                     scale=rstd[:, j:j + 1], bias=q4[:, j:j + 1])
            if lnab is not None:
                lg = lnab[:, 0:D][:, None, :].to_broadcast([P, nb, D])
                lb = lnab[:, D:2 * D][:, None, :].to_broadcast([P, nb, D])
                yf3 = yf[:, :nb, :]
                nc.vector.tensor_mul(out=yf3, in0=yf3, in1=lg)
                nc.vector.tensor_add(out=yf3, in0=yf3, in1=lb)
            n0 = t0 * P
            dst = d_y[n0:n0 + nb * P, :].rearrange("(t p) d -> p t d", p=P)
            nc.sync.dma_start(dst, yf[:, :nb, :])

        for gg in range(NG):
            tg0 = gg * G
            # msg layout: [P, q, tile-in-group, c, D] -- q outermost so each
            # chunk's gather writes one contiguous [P, G*Cq, D] section
            msg = msgp.tile([P, NCHK * G * Cq * D], bf, tag="msg", name="msg")
            # split each chunk's gather in two on separate swdge queues so the
            # Q7 descriptor-gen of one half overlaps the other half's drain
            nrow = G * Cq                   # msg rows per (chunk, group)
            h3 = (nrow + 2) // 3
            splits = ((0, h3), (h3, 2 * h3), (2 * h3, nrow))
            for q in range(NCHK):
                icol = (q * NG + gg) * (IPG // 16)
                for h, (r0, r1) in enumerate(splits):
                    nh = (r1 - r0) * P
                    sec = msg[:, (q * nrow + r0) * D:(q * nrow + r1) * D]
                    nc.gpsimd.dma_gather(
                        out_ap=sec.rearrange("p (s d) -> p s d", d=D),
                        in_ap=d_gq[q][:],
                        idxs_ap=idxs[:, icol + r0 * 8:icol + r0 * 8 + nh // 16],
                        num_idxs=nh,
                        num_idxs_reg=nh,
                        elem_size=D,
                        single_packet=False,
                        queue_num=2 * q + (h & 1),
                    )
            for tl in range(G):
                t = tg0 + tl
                j = t % B
                if j == 0:
                    nb = min(B, T - t)
                    g4 = t // B
                    bt = (apsp.tile([P, B * D], f32, space="PSUM", tag="aps4",
                                    name="aps4"),
                          ratp.tile([P, B * D], f32, space="PSUM", tag="rps4",
                                    name="rps4"),
                          gamp.tile([P, B * D], f32, space="PSUM", tag="gps4",
                                    name="gps4"),
                          eltp.tile([P, B, D], f32, tag="rate4", name="rate4"))
                    # group bias matmuls open the psum banks (start=True zeroes
                    # the WHOLE 2KB bank, so it must come before any slice
                    # accumulation): agg += 2*cnt (x) b_fc ; gamma += b_rob
                    nc.tensor.matmul(out=bt[0][:, :nb * D],
                                     lhsT=cnt4[:, g4 * P:(g4 + 1) * P],
                                     rhs=bfc4[:, :nb * D],
                                     start=True, stop=False)
                    nc.tensor.matmul(out=bt[2][:, :nb * D],
                                     lhsT=onesr[:], rhs=brobt[:, :nb * D],
                                     start=True, stop=False)
                aps4, rps4, gps4, rate4 = bt
                last_of_group = (j == B - 1 or t == T - 1)
                sel = selp.tile([P, SELW], bf, tag="sel", name="sel")
                rb = rowsr[:, t * NCHK * Cq:(t + 1) * NCHK * Cq][:, :, None] \
                    .to_broadcast([P, NCHK * Cq, P])
                nc.vector.tensor_tensor(
                    out=sel.rearrange("p (c m) -> p c m", c=NCHK * Cq), in0=rb,
                    in1=iota.rearrange("p (c m) -> p c m", c=NCHK * Cq),
                    op=mybir.AluOpType.is_equal)
                # agg slice: self term (host-prescaled cnt*x) + neighbor sums
                nc.tensor.matmul(out=aps4[:, j * D:(j + 1) * D],
                                 lhsT=xls[:, t * P:(t + 1) * P], rhs=wfc[:],
                                 start=False, stop=False)
                for q in range(NCHK):
                    for c in range(Cq):
                        cc = q * Cq + c
                        moff = ((q * G + tl) * Cq + c) * D
                        nc.tensor.matmul(
                            out=aps4[:, j * D:(j + 1) * D],
                            lhsT=sel[:, cc * P:(cc + 1) * P],
                            rhs=msg[:, moff:moff + D],
                            start=False,
                            stop=(last_of_group and q == NCHK - 1
                                  and c == Cq - 1))
                # rate / gamma GEMMs for this tile
                nc.tensor.matmul(out=rps4[:, j * D:(j + 1) * D],
                                 lhsT=xloc[:, t * P:(t + 1) * P], rhs=wrt[:],
                                 start=True, stop=True)
                nc.tensor.matmul(out=gps4[:, j * D:(j + 1) * D],
                                 lhsT=xloc[:, t * P:(t + 1) * P], rhs=wrb[:],
                                 start=False, stop=last_of_group)
                if last_of_group:
                    nb = j + 1
                    # softplus = ln(exp(z) + 1) on the ACT engine, per group
                    spt = eltp.tile([P, B, D], f32, tag="spt", name="spt")
                    r4v = rps4[:, :nb * D].rearrange("p (b d) -> p b d", d=D)
                    nc.scalar.activation(out=spt[:, :nb, :], in_=r4v,
                                         func=AF.Exp)
                    nc.scalar.activation(out=rate4[:, :nb, :],
                                         in_=spt[:, :nb, :],
                                         func=AF.Ln, bias=1.0)
                    eltwise(bt, t - j, nb)

    nc.compile()
    return nc


def run(inputs, cfg: Cfg, core_ids=None):
    in_maps = prep(**inputs, cfg=cfg)
    nc = build(cfg)
    res = run_bass_kernel_spmd(nc, in_maps, core_ids=core_ids or list(range(cfg.NC)))
    ys = [res.results[r]["y"][:cfg.NLOC] for r in range(cfg.NC)]
    return np.concatenate(ys, axis=0)


def kernel(**inputs):
    cfg = Cfg(N=100_000, E=800_000, NC=8)
    return run(inputs, cfg)


# revision 17
# speedup vs baseline: 1.0600x; 1.0600x over previous
"""Trainium2 Bass kernel for a GNN message-passing layer (BoundaryConvLayer).

Computation (reference, per node i over D=128 channels):
    rate  = softplus(x @ W_rate) + EPS
    gamma = x @ W_rob + b_rob
    h     = x @ W_fc + b_fc
    agg   = segment_sum(h[row] + h[col], row)
    y     = LayerNorm((rate*agg + gamma) / (1 + rate*deg + EPS)) * ln_gamma + ln_beta

Distribution: nodes sharded across 8 cores by contiguous row blocks; edges
partitioned by destination row so the segment sum is local.  Each core GEMMs
only the x rows its edges actually reference (host-side dedup, ~63k of 100k)
into a per-core DRAM gather table g = x_src @ W_fc; per-edge g[col] rows are
fetched with batched DMAGatherAnt.

Key identities:
    agg[i] = cnt[i]*g[i] + sum_{e:row=i} g[col_e] + 2*cnt[i]*b_fc
  where cnt = in-edge count (h = g + b_fc).  The neighbor sum is a one-hot
  "selection matrix" matmul on the PE over gathered edge rows; the self term
  cnt[i]*g[i] is one matmul with host-prescaled lhsT (cnt*x); both bias
  rank-1 terms are K<=4 matmuls issued once per 4-tile eltwise group.

Gather layout: dma_gather indices are int16, so the g table is split into
NCHK=2 chunks of CPAD rows (< 32768); per (tile, chunk) the edges fill Cq
128-slot groups (Cq = global max), zero-row pads in unused slots.  One dense
dma_gather per (chunk, 7-tile group).

LayerNorm: mean/var via bn_stats/bn_aggr on DVE, 1/den and rstd via
exp(-ln(.)) on the ACT engine (one activation table), final (y-mean)*rstd
as a single ACT Identity with per-partition scale/bias APs.
"""

import numpy as np
import ml_dtypes
from contextlib import ExitStack
from dataclasses import dataclass

import concourse.bass as bass
import concourse.tile as tile
from concourse import bacc, mybir
from concourse.bass_utils import run_bass_kernel_spmd

# The stock ACT-table chooser greedily picks the first set containing each
# function, which can alternate between sets and reload the table (~1.3us
# each).  Restrict it to the one set that contains all funcs we use
# (Exp, Ln, Copy, Identity, Square) so a single load suffices.
_ACT_KEEP = "natural_log_exp_and_others"
if not getattr(bacc, "_act_tables_patched", False):
    _orig_get_tables = bacc.get_activation_tables

    def _patched_get_tables(arch):
        t = _orig_get_tables(arch)
        if _ACT_KEEP in t:
            t = {k: (v if k == _ACT_KEEP else set()) for k, v in t.items()}
        return t

    bacc.get_activation_tables = _patched_get_tables
    bacc._act_tables_patched = True

BF16 = ml_dtypes.bfloat16
EPS = 1e-4
LN_EPS = 1e-5
P = 128
D = 128


@dataclass
class Cfg:
    N: int            # total nodes
    E: int            # total edges
    NC: int           # cores
    NCHK: int = 2     # gather table chunks (int16 range)
    CPAD: int = 32000  # padded chunk rows (128-aligned, <= 32767)
    Cq: int = 0       # 128-slot groups per (tile, chunk); set by prep
    ln_trivial: bool = False

    @property
    def NLOC(self):
        return self.N // self.NC

    @property
    def T(self):
        return (self.NLOC + P - 1) // P

    @property
    def TLP(self):
        return self.T * P

    @property
    def RPCC(self):   # real rows per chunk (>=64 zero pad rows at chunk end)
        return self.CPAD - 64

    @property
    def G(self):      # tiles per gather group
        for g in (7, 14, 4, 2, 1):
            if self.T % g == 0:
                return g
        return 1

    @property
    def B(self):      # tiles per eltwise group
        return 4


def prep(x, edge_index, degree, W_fc, b_fc, W_rate, W_rob, b_rob, ln_gamma, ln_beta,
         cfg: Cfg):
    """Host-side preprocessing: shard + build per-core gather/selection tables."""
    N, NC, NCHK, CPAD = cfg.N, cfg.NC, cfg.NCHK, cfg.CPAD
    NLOC, T, TLP, RPCC = cfg.NLOC, cfg.T, cfg.TLP, cfg.RPCC
    B = cfg.B
    NG4 = (T + B - 1) // B

    x = np.asarray(x, np.float32)
    edge_index = np.asarray(edge_index, np.int64)
    degree = np.asarray(degree)
    row, col = edge_index[0], edge_index[1]

    w_fc = np.ascontiguousarray(W_fc, dtype=np.float32).astype(BF16)
    w_rt = np.ascontiguousarray(W_rate, dtype=np.float32).astype(BF16)
    w_rb = np.ascontiguousarray(W_rob, dtype=np.float32).astype(BF16)

    bfc4 = np.zeros((B, B * D), BF16)       # block-diag b_fc (rhs of K=4 bias)
    for j in range(B):
        bfc4[j, j * D:(j + 1) * D] = np.asarray(b_fc, np.float32).astype(BF16)
    brobt = np.tile(np.asarray(b_rob, np.float32).astype(BF16)[None, :], (1, B))
    onesr = np.ones((1, P), BF16)

    cfg.ln_trivial = bool(np.all(np.asarray(ln_gamma) == 1.0)
                          and np.all(np.asarray(ln_beta) == 0.0))
    lnab = np.zeros((P, 2 * D), np.float32)
    lnab[:, :D] = np.asarray(ln_gamma, np.float32)[None, :]
    lnab[:, D:] = np.asarray(ln_beta, np.float32)[None, :]

    core_of = row // NLOC

    # pass 1: per-core dedup + per-(tile,chunk) counts fix the global Cq
    percore = []
    maxslots = 0
    for r in range(NC):
        m = core_of == r
        rl = (row[m] - r * NLOC).astype(np.int64)
        ce = col[m]
        srcs, inv = np.unique(ce, return_inverse=True)
        S = len(srcs)
        assert S <= NCHK * RPCC, (S, NCHK * RPCC)
        cq = inv // RPCC
        crow = inv - cq * RPCC
        cnt = np.bincount(rl, minlength=TLP)
        tq = (rl // P) * NCHK + cq
        cnt_tq = np.bincount(tq, minlength=T * NCHK).reshape(T, NCHK)
        maxslots = max(maxslots, int(cnt_tq.max()))
        percore.append((rl, cq, crow, cnt, cnt_tq, srcs))
    Cq = max(1, -(-maxslots // P))
    cfg.Cq = Cq
    G = cfg.G
    NG = T // G
    IPG = G * Cq * P           # idxs per (chunk, group) instruction

    in_maps = []
    for r in range(NC):
        rl, cq, crow, cnt, cnt_tq, srcs = percore[r]
        # order edges by (tile, chunk, src-row): dense grids + HBM locality
        order = np.lexsort((crow, cq, rl // P))
        rl_s, cq_s, crow_s = rl[order], cq[order], crow[order]
        t_s = rl_s // P
        tq_s = t_s * NCHK + cq_s
        run_start = np.zeros(T * NCHK + 1, np.int64)
        np.cumsum(cnt_tq.reshape(-1), out=run_start[1:])
        pos = np.arange(len(rl_s)) - run_start[tq_s]
        tl_s = t_s % G
        gg_s = t_s // G
        ipos = tl_s * (Cq * P) + pos
        idx16 = np.full((NCHK, NG, IPG), CPAD - 1, np.int16)  # pad -> zero row
        idx16[cq_s, gg_s, ipos] = crow_s.astype(np.int16)
        # wrap each stream: idx i -> [i%16, i//16], replicate to 128 partitions
        idxw = idx16.reshape(NCHK, NG, IPG // 16, 16).transpose(0, 1, 3, 2)
        idxw = np.ascontiguousarray(idxw)
        idxw = np.tile(idxw, (1, 1, 8, 1))           # [NCHK, NG, 128, IPG//16]
        idx_sb = np.ascontiguousarray(
            idxw.transpose(2, 0, 1, 3)).reshape(P, NCHK * NG * (IPG // 16))

        # rowsr: rebased row (node % 128) per slot, -1 for pads
        rowsr = np.full((P, T * NCHK * Cq), -1.0, BF16)
        slot_col = t_s * (NCHK * Cq) + cq_s * Cq + pos // P
        rowsr[pos % P, slot_col] = (rl_s % P).astype(BF16)

        iotab = np.ascontiguousarray(
            np.tile(np.arange(P, dtype=BF16)[None, :], (P, NCHK * Cq)))

        # dense source x, chunk-major, transposed; zero rows at chunk tails
        xs = np.zeros((NCHK * CPAD, D), np.float32)
        for q in range(NCHK):
            seg = srcs[q * RPCC:(q + 1) * RPCC]
            if len(seg):
                xs[q * CPAD:q * CPAD + len(seg)] = x[seg]
        XS = np.ascontiguousarray(xs.T.astype(BF16))

        xl = x[r * NLOC:(r + 1) * NLOC]
        xloc = np.zeros((P, TLP), BF16)
        xloc[:, :NLOC] = xl.T.astype(BF16)
        xls = np.zeros((P, TLP), BF16)
        xls[:, :NLOC] = (xl * cnt[:NLOC, None]).T.astype(BF16)

        cnt4 = np.zeros((B, NG4 * P), BF16)   # lhsT of the K=4 cnt*b_fc bias
        c2 = (2.0 * cnt).astype(np.float32)
        for g in range(NG4):
            for k in range(B):
                t = g * B + k
                if t < T:
                    cnt4[k, g * P:(g + 1) * P] = c2[t * P:(t + 1) * P].astype(BF16)

        degl = np.zeros(TLP, np.float32)
        degl[:NLOC] = degree[r * NLOC:(r + 1) * NLOC].astype(np.float32)
        degf = np.ascontiguousarray(degl.reshape(T, P).T)

        in_maps.append({
            "XS": XS, "xloc": xloc, "xls": xls,
            "Wfc": w_fc, "Wrt": w_rt, "Wrb": w_rb,
            "bfc4": bfc4, "brobt": brobt, "onesr": onesr, "lnab": lnab,
            "iotab": iotab, "rowsr": rowsr, "idxs": idx_sb,
            "cnt4": cnt4, "degf": degf,
        })
    return in_maps


def build(cfg: Cfg):
    """Build the SPMD Bass program (identical on every core)."""
    NC, T, TLP = cfg.NC, cfg.T, cfg.TLP
    NCHK, Cq, CPAD = cfg.NCHK, cfg.Cq, cfg.CPAD
    G, B = cfg.G, cfg.B
    NG = T // G
    NG4 = (T + B - 1) // B
    IPG = G * Cq * P
    SELW = NCHK * Cq * P       # sel width per tile
    bf = mybir.dt.bfloat16
    f32 = mybir.dt.float32
    i16 = mybir.dt.int16
    AF = mybir.ActivationFunctionType

    nc = bacc.Bacc("TRN2", target_bir_lowering=False, debug=False, num_devices=NC,
                   num_swdge_queues=4)
    for cval in (LN_EPS, 1.0 + EPS):
        cs = nc.alloc_sbuf_tensor(f"const-float32-{cval}", [P, 1], f32)
        nc.gpsimd.memset(cs.ap(), cval)
        nc.const_aps.aps[(f32, cval)] = cs.ap()
    nc.all_engine_barrier()

    d_XS = nc.dram_tensor("XS", [P, NCHK * CPAD], bf, kind="ExternalInput").ap()
    d_xloc = nc.dram_tensor("xloc", [P, TLP], bf, kind="ExternalInput").ap()
    d_xls = nc.dram_tensor("xls", [P, TLP], bf, kind="ExternalInput").ap()
    d_wfc = nc.dram_tensor("Wfc", [P, D], bf, kind="ExternalInput").ap()
    d_wrt = nc.dram_tensor("Wrt", [P, D], bf, kind="ExternalInput").ap()
    d_wrb = nc.dram_tensor("Wrb", [P, D], bf, kind="ExternalInput").ap()
    d_bfc4 = nc.dram_tensor("bfc4", [B, B * D], bf, kind="ExternalInput").ap()
    d_brobt = nc.dram_tensor("brobt", [1, B * D], bf, kind="ExternalInput").ap()
    d_ones = nc.dram_tensor("onesr", [1, P], bf, kind="ExternalInput").ap()
    d_lnab = nc.dram_tensor("lnab", [P, 2 * D], f32, kind="ExternalInput").ap()
    d_iota = nc.dram_tensor("iotab", [P, SELW], bf, kind="ExternalInput").ap()
    d_rowsr = nc.dram_tensor("rowsr", [P, T * NCHK * Cq], bf,
                             kind="ExternalInput").ap()
    d_idxs = nc.dram_tensor("idxs", [P, NCHK * NG * (IPG // 16)], i16,
                            kind="ExternalInput").ap()
    d_cnt4 = nc.dram_tensor("cnt4", [B, NG4 * P], bf, kind="ExternalInput").ap()
    d_degf = nc.dram_tensor("degf", [P, T], f32, kind="ExternalInput").ap()
    # one g-table tensor per chunk so chunk-q gathers depend only on chunk-q
    # phase-1 writes
    d_gq = [nc.dram_tensor(f"gtab{q}", [CPAD, D], bf, kind="Internal").ap()
            for q in range(NCHK)]
    d_y = nc.dram_tensor("y", [TLP, D], f32, kind="ExternalOutput").ap()

    with tile.TileContext(nc) as tc, ExitStack() as ctx:
        from concourse import library_config
        nc.gpsimd.load_library(library_config.mlp)
        consts = ctx.enter_context(tc.tile_pool(name="consts", bufs=1))
        wfc = consts.tile([P, D], bf)
        nc.sync.dma_start(wfc[:], d_wfc[:])
        # phase-3-only consts: allocate now, DMA after phase-1 issue so the
        # loads overlap the table GEMM instead of delaying it
        wrt = consts.tile([P, D], bf)
        wrb = consts.tile([P, D], bf)
        iota = consts.tile([P, SELW], bf)
        rowsr = consts.tile([P, T * NCHK * Cq], bf)
        idxs = consts.tile([P, NCHK * NG * (IPG // 16)], i16)
        cnt4 = consts.tile([B, NG4 * P], bf)
        bfc4 = consts.tile([B, B * D], bf)
        brobt = consts.tile([1, B * D], bf)
        onesr = consts.tile([1, P], bf)
        degf = consts.tile([P, T], f32)
        xloc = consts.tile([P, TLP], bf)
        xls = consts.tile([P, TLP], bf)
        lnab = consts.tile([P, 2 * D], f32) if not cfg.ln_trivial else None

        def load_phase3_consts():
            nc.sync.dma_start(wrt[:], d_wrt[:])
            nc.sync.dma_start(wrb[:], d_wrb[:])
            nc.sync.dma_start(iota[:], d_iota[:])
            nc.sync.dma_start(rowsr[:], d_rowsr[:])
            nc.sync.dma_start(idxs[:], d_idxs[:])
            nc.sync.dma_start(cnt4[:], d_cnt4[:])
            nc.sync.dma_start(bfc4[:], d_bfc4[:])
            nc.sync.dma_start(brobt[:], d_brobt[:])
            nc.sync.dma_start(onesr[:], d_ones[:])
            nc.sync.dma_start(degf[:], d_degf[:])
            nc.sync.dma_start(xloc[:], d_xloc[:])
            nc.sync.dma_start(xls[:], d_xls[:])
            if lnab is not None:
                nc.sync.dma_start(lnab[:], d_lnab[:])

        # phase-3 psum pools first: disjoint banks from phase-1's pool
        apsp = ctx.enter_context(tc.tile_pool(name="apsp", bufs=2, space="PSUM"))
        ratp = ctx.enter_context(tc.tile_pool(name="ratp", bufs=2, space="PSUM"))
        gamp = ctx.enter_context(tc.tile_pool(name="gamp", bufs=2, space="PSUM"))
        msgp = ctx.enter_context(tc.tile_pool(name="msgp", bufs=3))
        selp = ctx.enter_context(tc.tile_pool(name="selp", bufs=8))
        eltp = ctx.enter_context(tc.tile_pool(name="eltp", bufs=2))
        smallp = ctx.enter_context(tc.tile_pool(name="smallp", bufs=2))

        # ---------------- phase 1: g = x_src @ W_fc (dedup'd) ----------------
        CHUNK = 4096
        GRP = 512
        with tc.tile_pool(name="p1x", bufs=2) as p1x, \
             tc.tile_pool(name="p1ps", bufs=2, space="PSUM") as p1ps, \
             tc.tile_pool(name="p1st", bufs=6) as p1st:
            ng = 0
            # interleave the two chunks so both gather tables complete
            # (almost) simultaneously -- chunk-1 gathers are on the critical
            # path and must not wait for a sequential full phase 1
            for c0 in range(0, CPAD, CHUNK):
                for q in range(NCHK):
                    cw = min(CHUNK, CPAD - c0)
                    xc = p1x.tile([P, CHUNK], bf, tag="xc", name="xc")
                    nc.sync.dma_start(xc[:, :cw],
                                      d_XS[:, q * CPAD + c0:q * CPAD + c0 + cw])
                    for g0 in range(0, cw, GRP):
                        gw = min(GRP, cw - g0)
                        gps = p1ps.tile([P, GRP], f32, space="PSUM", tag="gps",
                                        name="gps")
                        for j in range(0, gw, P):
                            nc.tensor.matmul(
                                out=gps[:, j:j + P],
                                lhsT=xc[:, g0 + j:g0 + j + P],
                                rhs=wfc[:],
                                start=True, stop=True,
                            )
                        gst = p1st.tile([P, GRP], bf, tag="gst", name="gst")
                        # alternate the psum->bf16 conversion between the two
                        # free elementwise engines so neither paces phase 1
                        if ng & 1:
                            nc.scalar.copy(gst[:, :gw], gps[:, :gw])
                        else:
                            nc.vector.tensor_copy(gst[:, :gw], gps[:, :gw])
                        ng += 1
                        dst = d_gq[q][c0 + g0:c0 + g0 + gw, :].rearrange(
                            "(t p) d -> p t d", p=P)
                        nc.sync.dma_start(dst, gst[:, :gw].rearrange(
                            "p (t d) -> p t d", d=D))

        load_phase3_consts()

        # ---------------- phase 3: message passing + elementwise -------------
        bt = None

        def eltwise(bt, t0, nb):
            aps4, rps4, gps4, rate4 = bt
            a3 = aps4[:, :nb * D].rearrange("p (b d) -> p b d", d=D)
            g3 = gps4[:, :nb * D].rearrange("p (b d) -> p b d", d=D)
            r3 = rate4[:, :nb, :]
            num = eltp.tile([P, B, D], f32, tag="num", name="num")[:, :nb, :]
            den = eltp.tile([P, B, D], f32, tag="den", name="den")[:, :nb, :]
            y0 = eltp.tile([P, B, D], f32, tag="y0", name="y0")
            yf = eltp.tile([P, B, D], f32, tag="yf", name="yf")
            st6 = smallp.tile([P, B, 6], f32, tag="st6", name="st6")
            st2 = smallp.tile([P, B, 2], f32, tag="st2", name="st2")
            sm = smallp.tile([P, 4 * B], f32, tag="sm", name="sm")
            lnv = sm[:, 0:nb]
            rstd = sm[:, B:B + nb]
            q4 = sm[:, 2 * B:2 * B + nb]
            y03 = y0[:, :nb, :]

            # num = (softplus + EPS) * agg + gamma
            nc.vector.scalar_tensor_tensor(
                out=num, in0=r3, scalar=EPS, in1=a3,
                op0=mybir.AluOpType.add, op1=mybir.AluOpType.mult)
            nc.vector.tensor_add(out=num, in0=num, in1=g3)
            # den = (softplus + EPS) * deg;  1/(den + 1 + EPS) = exp(-ln(.))
            degb = degf[:, t0:t0 + nb][:, :, None].to_broadcast([P, nb, D])
            nc.vector.scalar_tensor_tensor(
                out=den, in0=r3, scalar=EPS, in1=degb,
                op0=mybir.AluOpType.add, op1=mybir.AluOpType.mult)
            nc.scalar.activation(out=den, in_=den, func=AF.Ln, bias=1.0 + EPS)
            nc.scalar.activation(out=den, in_=den, func=AF.Exp, scale=-1.0)
            nc.vector.tensor_mul(out=y03, in0=num, in1=den)
            # LayerNorm stats: bn_stats/bn_aggr give per-tile (mean, var)
            for j in range(nb):
                nc.vector.bn_stats(out=st6[:, j, :], in_=y0[:, j, :])
                nc.vector.bn_aggr(out=st2[:, j, :], in_=st6[:, j, :])
            nc.scalar.activation(out=lnv, in_=st2[:, :nb, 1], func=AF.Ln,
                                 bias=LN_EPS)
            nc.scalar.activation(out=rstd, in_=lnv, func=AF.Exp, scale=-0.5)
            # q4 = -mean * rstd;  yf = y0 * rstd + q4  (per-partition ACT affine)
            nc.vector.scalar_tensor_tensor(
                out=q4, in0=st2[:, :nb, 0], scalar=-1.0, in1=rstd,
                op0=mybir.AluOpType.mult, op1=mybir.AluOpType.mult)
            for j in range(nb):
                nc.scalar.activation(out=yf[:, j, :], in_=y0[:, j, :],
                                     func=AF.Identity,
                                     scale=rstd[:, j:j + 1], bias=q4[:, j:j + 1])
            if lnab is not None:
                lg = lnab[:, 0:D][:, None, :].to_broadcast([P, nb, D])
                lb = lnab[:, D:2 * D][:, None, :].to_broadcast([P, nb, D])
                yf3 = yf[:, :nb, :]
                nc.vector.tensor_mul(out=yf3, in0=yf3, in1=lg)
                nc.vector.tensor_add(out=yf3, in0=yf3, in1=lb)
            n0 = t0 * P
            dst = d_y[n0:n0 + nb * P, :].rearrange("(t p) d -> p t d", p=P)
            nc.sync.dma_start(dst, yf[:, :nb, :])

        for gg in range(NG):
            tg0 = gg * G
            # msg layout: [P, q, tile-in-group, c, D] -- q outermost so each
            # chunk's gather writes one contiguous [P, G*Cq, D] section
            msg = msgp.tile([P, NCHK * G * Cq * D], bf, tag="msg", name="msg")
            # split each chunk's gather in two on separate swdge queues so the
            # Q7 descriptor-gen of one half overlaps the other half's drain
            nrow = G * Cq                   # msg rows per (chunk, group)
            hrow = (nrow + 1) // 2
            for q in range(NCHK):
                icol = (q * NG + gg) * (IPG // 16)
                for h, (r0, r1) in enumerate(((0, hrow), (hrow, nrow))):
                    nh = (r1 - r0) * P
                    sec = msg[:, (q * nrow + r0) * D:(q * nrow + r1) * D]
                    nc.gpsimd.dma_gather(
                        out_ap=sec.rearrange("p (s d) -> p s d", d=D),
                        in_ap=d_gq[q][:],
                        idxs_ap=idxs[:, icol + r0 * 8:icol + r0 * 8 + nh // 16],
                        num_idxs=nh,
                        num_idxs_reg=nh,
                        elem_size=D,
                        single_packet=False,
                        queue_num=2 * q + h,
                    )
            for tl in range(G):
                t = tg0 + tl
                j = t % B
                if j == 0:
                    nb = min(B, T - t)
                    g4 = t // B
                    bt = (apsp.tile([P, B * D], f32, space="PSUM", tag="aps4",
                                    name="aps4"),
                          ratp.tile([P, B * D], f32, space="PSUM", tag="rps4",
                                    name="rps4"),
                          gamp.tile([P, B * D], f32, space="PSUM", tag="gps4",
                                    name="gps4"),
                          eltp.tile([P, B, D], f32, tag="rate4", name="rate4"))
                    # group bias matmuls open the psum banks (start=True zeroes
                    # the WHOLE 2KB bank, so it must come before any slice
                    # accumulation): agg += 2*cnt (x) b_fc ; gamma += b_rob
                    nc.tensor.matmul(out=bt[0][:, :nb * D],
                                     lhsT=cnt4[:, g4 * P:(g4 + 1) * P],
                                     rhs=bfc4[:, :nb * D],
                                     start=True, stop=False)
                    nc.tensor.matmul(out=bt[2][:, :nb * D],
                                     lhsT=onesr[:], rhs=brobt[:, :nb * D],
                                     start=True, stop=False)
                aps4, rps4, gps4, rate4 = bt
                last_of_group = (j == B - 1 or t == T - 1)
                sel = selp.tile([P, SELW], bf, tag="sel", name="sel")
                rb = rowsr[:, t * NCHK * Cq:(t + 1) * NCHK * Cq][:, :, None] \
                    .to_broadcast([P, NCHK * Cq, P])
                nc.vector.tensor_tensor(
                    out=sel.rearrange("p (c m) -> p c m", c=NCHK * Cq), in0=rb,
                    in1=iota.rearrange("p (c m) -> p c m", c=NCHK * Cq),
                    op=mybir.AluOpType.is_equal)
                # agg slice: self term (host-prescaled cnt*x) + neighbor sums
                nc.tensor.matmul(out=aps4[:, j * D:(j + 1) * D],
                                 lhsT=xls[:, t * P:(t + 1) * P], rhs=wfc[:],
                                 start=False, stop=False)
                for q in range(NCHK):
                    for c in range(Cq):
                        cc = q * Cq + c
                        moff = ((q * G + tl) * Cq + c) * D
                        nc.tensor.matmul(
                            out=aps4[:, j * D:(j + 1) * D],
                            lhsT=sel[:, cc * P:(cc + 1) * P],
                            rhs=msg[:, moff:moff + D],
                            start=False,
                            stop=(last_of_group and q == NCHK - 1
                                  and c == Cq - 1))
                # rate / gamma GEMMs for this tile
                nc.tensor.matmul(out=rps4[:, j * D:(j + 1) * D],
                                 lhsT=xloc[:, t * P:(t + 1) * P], rhs=wrt[:],
                                 start=True, stop=True)
                nc.tensor.matmul(out=gps4[:, j * D:(j + 1) * D],
                                 lhsT=xloc[:, t * P:(t + 1) * P], rhs=wrb[:],
                                 start=False, stop=last_of_group)
                if last_of_group:
                    nb = j + 1
                    # softplus = ln(exp(z) + 1) on the ACT engine, per group
                    spt = eltp.tile([P, B, D], f32, tag="spt", name="spt")
                    r4v = rps4[:, :nb * D].rearrange("p (b d) -> p b d", d=D)
                    nc.scalar.activation(out=spt[:, :nb, :], in_=r4v,
                                         func=AF.Exp)
                    nc.scalar.activation(out=rate4[:, :nb, :],
                                         in_=spt[:, :nb, :],
                                         func=AF.Ln, bias=1.0)
                    eltwise(bt, t - j, nb)

    nc.compile()
    return nc


def run(inputs, cfg: Cfg, core_ids=None):
    in_maps = prep(**inputs, cfg=cfg)
    nc = build(cfg)
    res = run_bass_kernel_spmd(nc, in_maps, core_ids=core_ids or list(range(cfg.NC)))
    ys = [res.results[r]["y"][:cfg.NLOC] for r in range(cfg.NC)]
    return np.concatenate(ys, axis=0)


def kernel(**inputs):
    cfg = Cfg(N=100_000, E=800_000, NC=8)
    return run(inputs, cfg)


# revision 18
# speedup vs baseline: 1.0854x; 1.0239x over previous
"""Trainium2 Bass kernel for a GNN message-passing layer (BoundaryConvLayer).

Computation (reference, per node i over D=128 channels):
    rate  = softplus(x @ W_rate) + EPS
    gamma = x @ W_rob + b_rob
    h     = x @ W_fc + b_fc
    agg   = segment_sum(h[row] + h[col], row)
    y     = LayerNorm((rate*agg + gamma) / (1 + rate*deg + EPS)) * ln_gamma + ln_beta

Distribution: nodes sharded across 8 cores by contiguous row blocks; edges
partitioned by destination row so the segment sum is local.  Each core GEMMs
only the x rows its edges actually reference (host-side dedup, ~63k of 100k)
into a per-core DRAM gather table g = x_src @ W_fc; per-edge g[col] rows are
fetched with batched DMAGatherAnt.

Key identities:
    agg[i] = cnt[i]*g[i] + sum_{e:row=i} g[col_e] + 2*cnt[i]*b_fc
  where cnt = in-edge count (h = g + b_fc).  The neighbor sum is a one-hot
  "selection matrix" matmul on the PE over gathered edge rows; the self term
  cnt[i]*g[i] is one matmul with host-prescaled lhsT (cnt*x); both bias
  rank-1 terms are K<=4 matmuls issued once per 4-tile eltwise group.

Gather layout: dma_gather indices are int16, so the g table is split into
NCHK=2 chunks of CPAD rows (< 32768); per (tile, chunk) the edges fill Cq
128-slot groups (Cq = global max), zero-row pads in unused slots.  One dense
dma_gather per (chunk, 7-tile group).

LayerNorm: mean/var via bn_stats/bn_aggr on DVE, 1/den and rstd via
exp(-ln(.)) on the ACT engine (one activation table), final (y-mean)*rstd
as a single ACT Identity with per-partition scale/bias APs.
"""

import numpy as np
import ml_dtypes
from contextlib import ExitStack
from dataclasses import dataclass

import concourse.bass as bass
import concourse.tile as tile
from concourse import bacc, mybir
from concourse.bass_utils import run_bass_kernel_spmd

# The stock ACT-table chooser greedily picks the first set containing each
# function, which can alternate between sets and reload the table (~1.3us
# each).  Restrict it to the one set that contains all funcs we use
# (Exp, Ln, Copy, Identity, Square) so a single load suffices.
_ACT_KEEP = "natural_log_exp_and_others"
if not getattr(bacc, "_act_tables_patched", False):
    _orig_get_tables = bacc.get_activation_tables

    def _patched_get_tables(arch):
        t = _orig_get_tables(arch)
        if _ACT_KEEP in t:
            t = {k: (v if k == _ACT_KEEP else set()) for k, v in t.items()}
        return t

    bacc.get_activation_tables = _patched_get_tables
    bacc._act_tables_patched = True

BF16 = ml_dtypes.bfloat16
EPS = 1e-4
LN_EPS = 1e-5
P = 128
D = 128


@dataclass
class Cfg:
    N: int            # total nodes
    E: int            # total edges
    NC: int           # cores
    NCHK: int = 2     # gather table chunks (int16 range)
    CPAD: int = 32000  # padded chunk rows (128-aligned, <= 32767)
    Cq: int = 0       # 128-slot groups per (tile, chunk); set by prep
    ln_trivial: bool = False

    @property
    def NLOC(self):
        return self.N // self.NC

    @property
    def T(self):
        return (self.NLOC + P - 1) // P

    @property
    def TLP(self):
        return self.T * P

    @property
    def RPCC(self):   # real rows per chunk (>=64 zero pad rows at chunk end)
        return self.CPAD - 64

    @property
    def G(self):      # tiles per gather group
        for g in (7, 14, 4, 2, 1):
            if self.T % g == 0:
                return g
        return 1

    @property
    def B(self):      # tiles per eltwise group
        return 4


def prep(x, edge_index, degree, W_fc, b_fc, W_rate, W_rob, b_rob, ln_gamma, ln_beta,
         cfg: Cfg):
    """Host-side preprocessing: shard + build per-core gather/selection tables."""
    N, NC, NCHK, CPAD = cfg.N, cfg.NC, cfg.NCHK, cfg.CPAD
    NLOC, T, TLP, RPCC = cfg.NLOC, cfg.T, cfg.TLP, cfg.RPCC
    B = cfg.B
    NG4 = (T + B - 1) // B

    x = np.asarray(x, np.float32)
    edge_index = np.asarray(edge_index, np.int64)
    degree = np.asarray(degree)
    row, col = edge_index[0], edge_index[1]

    w_fc = np.ascontiguousarray(W_fc, dtype=np.float32).astype(BF16)
    w_rt = np.ascontiguousarray(W_rate, dtype=np.float32).astype(BF16)
    w_rb = np.ascontiguousarray(W_rob, dtype=np.float32).astype(BF16)

    bfc4 = np.zeros((B, B * D), BF16)       # block-diag b_fc (rhs of K=4 bias)
    for j in range(B):
        bfc4[j, j * D:(j + 1) * D] = np.asarray(b_fc, np.float32).astype(BF16)
    brobt = np.tile(np.asarray(b_rob, np.float32).astype(BF16)[None, :], (1, B))
    onesr = np.ones((1, P), BF16)

    cfg.ln_trivial = bool(np.all(np.asarray(ln_gamma) == 1.0)
                          and np.all(np.asarray(ln_beta) == 0.0))
    lnab = np.zeros((P, 2 * D), np.float32)
    lnab[:, :D] = np.asarray(ln_gamma, np.float32)[None, :]
    lnab[:, D:] = np.asarray(ln_beta, np.float32)[None, :]

    core_of = row // NLOC

    # pass 1: per-core dedup + per-(tile,chunk) counts fix the global Cq
    percore = []
    maxslots = 0
    for r in range(NC):
        m = core_of == r
        rl = (row[m] - r * NLOC).astype(np.int64)
        ce = col[m]
        srcs, inv = np.unique(ce, return_inverse=True)
        S = len(srcs)
        assert S <= NCHK * RPCC, (S, NCHK * RPCC)
        cq = inv // RPCC
        crow = inv - cq * RPCC
        cnt = np.bincount(rl, minlength=TLP)
        tq = (rl // P) * NCHK + cq
        cnt_tq = np.bincount(tq, minlength=T * NCHK).reshape(T, NCHK)
        maxslots = max(maxslots, int(cnt_tq.max()))
        percore.append((rl, cq, crow, cnt, cnt_tq, srcs))
    Cq = max(1, -(-maxslots // P))
    cfg.Cq = Cq
    G = cfg.G
    NG = T // G
    IPG = G * Cq * P           # idxs per (chunk, group) instruction

    in_maps = []
    for r in range(NC):
        rl, cq, crow, cnt, cnt_tq, srcs = percore[r]
        # order edges by (tile, chunk, src-row): dense grids + HBM locality
        order = np.lexsort((crow, cq, rl // P))
        rl_s, cq_s, crow_s = rl[order], cq[order], crow[order]
        t_s = rl_s // P
        tq_s = t_s * NCHK + cq_s
        run_start = np.zeros(T * NCHK + 1, np.int64)
        np.cumsum(cnt_tq.reshape(-1), out=run_start[1:])
        pos = np.arange(len(rl_s)) - run_start[tq_s]
        tl_s = t_s % G
        gg_s = t_s // G
        ipos = tl_s * (Cq * P) + pos
        idx16 = np.full((NCHK, NG, IPG), CPAD - 1, np.int16)  # pad -> zero row
        idx16[cq_s, gg_s, ipos] = crow_s.astype(np.int16)
        # wrap each stream: idx i -> [i%16, i//16], replicate to 128 partitions
        idxw = idx16.reshape(NCHK, NG, IPG // 16, 16).transpose(0, 1, 3, 2)
        idxw = np.ascontiguousarray(idxw)
        idxw = np.tile(idxw, (1, 1, 8, 1))           # [NCHK, NG, 128, IPG//16]
        idx_sb = np.ascontiguousarray(
            idxw.transpose(2, 0, 1, 3)).reshape(P, NCHK * NG * (IPG // 16))

        # rowsr: rebased row (node % 128) per slot, -1 for pads
        rowsr = np.full((P, T * NCHK * Cq), -1.0, BF16)
        slot_col = t_s * (NCHK * Cq) + cq_s * Cq + pos // P
        rowsr[pos % P, slot_col] = (rl_s % P).astype(BF16)

        iotab = np.ascontiguousarray(
            np.tile(np.arange(P, dtype=BF16)[None, :], (P, NCHK * Cq)))

        # dense source x, chunk-major, transposed; zero rows at chunk tails
        xs = np.zeros((NCHK * CPAD, D), np.float32)
        for q in range(NCHK):
            seg = srcs[q * RPCC:(q + 1) * RPCC]
            if len(seg):
                xs[q * CPAD:q * CPAD + len(seg)] = x[seg]
        XS = np.ascontiguousarray(xs.T.astype(BF16))

        xl = x[r * NLOC:(r + 1) * NLOC]
        xloc = np.zeros((P, TLP), BF16)
        xloc[:, :NLOC] = xl.T.astype(BF16)
        xls = np.zeros((P, TLP), BF16)
        xls[:, :NLOC] = (xl * cnt[:NLOC, None]).T.astype(BF16)

        cnt4 = np.zeros((B, NG4 * P), BF16)   # lhsT of the K=4 cnt*b_fc bias
        c2 = (2.0 * cnt).astype(np.float32)
        for g in range(NG4):
            for k in range(B):
                t = g * B + k
                if t < T:
                    cnt4[k, g * P:(g + 1) * P] = c2[t * P:(t + 1) * P].astype(BF16)

        degl = np.zeros(TLP, np.float32)
        degl[:NLOC] = degree[r * NLOC:(r + 1) * NLOC].astype(np.float32)
        degf = np.ascontiguousarray(degl.reshape(T, P).T)

        in_maps.append({
            "XS": XS, "xloc": xloc, "xls": xls,
            "Wfc": w_fc, "Wrt": w_rt, "Wrb": w_rb,
            "bfc4": bfc4, "brobt": brobt, "onesr": onesr, "lnab": lnab,
            "iotab": iotab, "rowsr": rowsr, "idxs": idx_sb,
            "cnt4": cnt4, "degf": degf,
        })
    return in_maps


def build(cfg: Cfg):
    """Build the SPMD Bass program (identical on every core)."""
    NC, T, TLP = cfg.NC, cfg.T, cfg.TLP
    NCHK, Cq, CPAD = cfg.NCHK, cfg.Cq, cfg.CPAD
    G, B = cfg.G, cfg.B
    NG = T // G
    NG4 = (T + B - 1) // B
    IPG = G * Cq * P
    SELW = NCHK * Cq * P       # sel width per tile
    bf = mybir.dt.bfloat16
    f32 = mybir.dt.float32
    i16 = mybir.dt.int16
    AF = mybir.ActivationFunctionType

    nc = bacc.Bacc("TRN2", target_bir_lowering=False, debug=False, num_devices=NC,
                   num_swdge_queues=4)
    for cval in (LN_EPS, 1.0 + EPS):
        cs = nc.alloc_sbuf_tensor(f"const-float32-{cval}", [P, 1], f32)
        nc.gpsimd.memset(cs.ap(), cval)
        nc.const_aps.aps[(f32, cval)] = cs.ap()
    nc.all_engine_barrier()

    d_XS = nc.dram_tensor("XS", [P, NCHK * CPAD], bf, kind="ExternalInput").ap()
    d_xloc = nc.dram_tensor("xloc", [P, TLP], bf, kind="ExternalInput").ap()
    d_xls = nc.dram_tensor("xls", [P, TLP], bf, kind="ExternalInput").ap()
    d_wfc = nc.dram_tensor("Wfc", [P, D], bf, kind="ExternalInput").ap()
    d_wrt = nc.dram_tensor("Wrt", [P, D], bf, kind="ExternalInput").ap()
    d_wrb = nc.dram_tensor("Wrb", [P, D], bf, kind="ExternalInput").ap()
    d_bfc4 = nc.dram_tensor("bfc4", [B, B * D], bf, kind="ExternalInput").ap()
    d_brobt = nc.dram_tensor("brobt", [1, B * D], bf, kind="ExternalInput").ap()
    d_ones = nc.dram_tensor("onesr", [1, P], bf, kind="ExternalInput").ap()
    d_lnab = nc.dram_tensor("lnab", [P, 2 * D], f32, kind="ExternalInput").ap()
    d_iota = nc.dram_tensor("iotab", [P, SELW], bf, kind="ExternalInput").ap()
    d_rowsr = nc.dram_tensor("rowsr", [P, T * NCHK * Cq], bf,
                             kind="ExternalInput").ap()
    d_idxs = nc.dram_tensor("idxs", [P, NCHK * NG * (IPG // 16)], i16,
                            kind="ExternalInput").ap()
    d_cnt4 = nc.dram_tensor("cnt4", [B, NG4 * P], bf, kind="ExternalInput").ap()
    d_degf = nc.dram_tensor("degf", [P, T], f32, kind="ExternalInput").ap()
    # one g-table tensor per chunk so chunk-q gathers depend only on chunk-q
    # phase-1 writes
    d_gq = [nc.dram_tensor(f"gtab{q}", [CPAD, D], bf, kind="Internal").ap()
            for q in range(NCHK)]
    d_y = nc.dram_tensor("y", [TLP, D], f32, kind="ExternalOutput").ap()

    with tile.TileContext(nc) as tc, ExitStack() as ctx:
        from concourse import library_config
        nc.gpsimd.load_library(library_config.mlp)
        consts = ctx.enter_context(tc.tile_pool(name="consts", bufs=1))
        wfc = consts.tile([P, D], bf)
        nc.sync.dma_start(wfc[:], d_wfc[:])
        wrt = consts.tile([P, D], bf)
        nc.sync.dma_start(wrt[:], d_wrt[:])
        wrb = consts.tile([P, D], bf)
        nc.sync.dma_start(wrb[:], d_wrb[:])
        iota = consts.tile([P, SELW], bf)
        nc.sync.dma_start(iota[:], d_iota[:])
        rowsr = consts.tile([P, T * NCHK * Cq], bf)
        nc.sync.dma_start(rowsr[:], d_rowsr[:])
        idxs = consts.tile([P, NCHK * NG * (IPG // 16)], i16)
        nc.sync.dma_start(idxs[:], d_idxs[:])
        cnt4 = consts.tile([B, NG4 * P], bf)
        nc.sync.dma_start(cnt4[:], d_cnt4[:])
        bfc4 = consts.tile([B, B * D], bf)
        nc.sync.dma_start(bfc4[:], d_bfc4[:])
        brobt = consts.tile([1, B * D], bf)
        nc.sync.dma_start(brobt[:], d_brobt[:])
        onesr = consts.tile([1, P], bf)
        nc.sync.dma_start(onesr[:], d_ones[:])
        degf = consts.tile([P, T], f32)
        nc.sync.dma_start(degf[:], d_degf[:])
        xloc = consts.tile([P, TLP], bf)
        nc.sync.dma_start(xloc[:], d_xloc[:])
        xls = consts.tile([P, TLP], bf)
        nc.sync.dma_start(xls[:], d_xls[:])
        lnab = None
        if not cfg.ln_trivial:
            lnab = consts.tile([P, 2 * D], f32)
            nc.sync.dma_start(lnab[:], d_lnab[:])

        # phase-3 psum pools first: disjoint banks from phase-1's pool
        apsp = ctx.enter_context(tc.tile_pool(name="apsp", bufs=2, space="PSUM"))
        ratp = ctx.enter_context(tc.tile_pool(name="ratp", bufs=2, space="PSUM"))
        gamp = ctx.enter_context(tc.tile_pool(name="gamp", bufs=2, space="PSUM"))
        msgp = ctx.enter_context(tc.tile_pool(name="msgp", bufs=3))
        selp = ctx.enter_context(tc.tile_pool(name="selp", bufs=8))
        eltp = ctx.enter_context(tc.tile_pool(name="eltp", bufs=2))
        smallp = ctx.enter_context(tc.tile_pool(name="smallp", bufs=2))

        # ---------------- phase 1: g = x_src @ W_fc (dedup'd) ----------------
        CHUNK = 4096
        GRP = 512
        with tc.tile_pool(name="p1x", bufs=2) as p1x, \
             tc.tile_pool(name="p1ps", bufs=2, space="PSUM") as p1ps, \
             tc.tile_pool(name="p1st", bufs=6) as p1st:
            ng = 0
            # interleave the two chunks so both gather tables complete
            # (almost) simultaneously -- chunk-1 gathers are on the critical
            # path and must not wait for a sequential full phase 1
            for c0 in range(0, CPAD, CHUNK):
                for q in range(NCHK):
                    cw = min(CHUNK, CPAD - c0)
                    xc = p1x.tile([P, CHUNK], bf, tag="xc", name="xc")
                    nc.sync.dma_start(xc[:, :cw],
                                      d_XS[:, q * CPAD + c0:q * CPAD + c0 + cw])
                    for g0 in range(0, cw, GRP):
                        gw = min(GRP, cw - g0)
                        gps = p1ps.tile([P, GRP], f32, space="PSUM", tag="gps",
                                        name="gps")
                        for j in range(0, gw, P):
                            nc.tensor.matmul(
                                out=gps[:, j:j + P],
                                lhsT=xc[:, g0 + j:g0 + j + P],
                                rhs=wfc[:],
                                start=True, stop=True,
                            )
                        gst = p1st.tile([P, GRP], bf, tag="gst", name="gst")
                        # alternate the psum->bf16 conversion between the two
                        # free elementwise engines so neither paces phase 1
                        if ng & 1:
                            nc.scalar.copy(gst[:, :gw], gps[:, :gw])
                        else:
                            nc.vector.tensor_copy(gst[:, :gw], gps[:, :gw])
                        ng += 1
                        dst = d_gq[q][c0 + g0:c0 + g0 + gw, :].rearrange(
                            "(t p) d -> p t d", p=P)
                        nc.sync.dma_start(dst, gst[:, :gw].rearrange(
                            "p (t d) -> p t d", d=D))

        # ---------------- phase 3: message passing + elementwise -------------
        bt = None

        def eltwise(bt, t0, nb):
            aps4, rps4, gps4, rate4 = bt
            a3 = aps4[:, :nb * D].rearrange("p (b d) -> p b d", d=D)
            g3 = gps4[:, :nb * D].rearrange("p (b d) -> p b d", d=D)
            r3 = rate4[:, :nb, :]
            num = eltp.tile([P, B, D], f32, tag="num", name="num")[:, :nb, :]
            den = eltp.tile([P, B, D], f32, tag="den", name="den")[:, :nb, :]
            y0 = eltp.tile([P, B, D], f32, tag="y0", name="y0")
            yf = eltp.tile([P, B, D], f32, tag="yf", name="yf")
            st6 = smallp.tile([P, B, 6], f32, tag="st6", name="st6")
            st2 = smallp.tile([P, B, 2], f32, tag="st2", name="st2")
            sm = smallp.tile([P, 4 * B], f32, tag="sm", name="sm")
            lnv = sm[:, 0:nb]
            rstd = sm[:, B:B + nb]
            q4 = sm[:, 2 * B:2 * B + nb]
            y03 = y0[:, :nb, :]

            # num = (softplus + EPS) * agg + gamma
            nc.vector.scalar_tensor_tensor(
                out=num, in0=r3, scalar=EPS, in1=a3,
                op0=mybir.AluOpType.add, op1=mybir.AluOpType.mult)
            nc.vector.tensor_add(out=num, in0=num, in1=g3)
            # den = (softplus + EPS) * deg;  1/(den + 1 + EPS) = exp(-ln(.))
            degb = degf[:, t0:t0 + nb][:, :, None].to_broadcast([P, nb, D])
            nc.vector.scalar_tensor_tensor(
                out=den, in0=r3, scalar=EPS, in1=degb,
                op0=mybir.AluOpType.add, op1=mybir.AluOpType.mult)
            nc.scalar.activation(out=den, in_=den, func=AF.Ln, bias=1.0 + EPS)
            nc.scalar.activation(out=den, in_=den, func=AF.Exp, scale=-1.0)
            nc.vector.tensor_mul(out=y03, in0=num, in1=den)
            # LayerNorm stats: bn_stats/bn_aggr give per-tile (mean, var)
            for j in range(nb):
                nc.vector.bn_stats(out=st6[:, j, :], in_=y0[:, j, :])
                nc.vector.bn_aggr(out=st2[:, j, :], in_=st6[:, j, :])
            nc.scalar.activation(out=lnv, in_=st2[:, :nb, 1], func=AF.Ln,
                                 bias=LN_EPS)
            nc.scalar.activation(out=rstd, in_=lnv, func=AF.Exp, scale=-0.5)
            # q4 = -mean * rstd;  yf = y0 * rstd + q4  (per-partition ACT affine)
            nc.vector.scalar_tensor_tensor(
                out=q4, in0=st2[:, :nb, 0], scalar=-1.0, in1=rstd,
                op0=mybir.AluOpType.mult, op1=mybir.AluOpType.mult)
            for j in range(nb):
                nc.scalar.activation(out=yf[:, j, :], in_=y0[:, j, :],
                                     func=AF.Identity,
                                     scale=rstd[:, j:j + 1], bias=q4[:, j:j + 1])
            if lnab is not None:
                lg = lnab[:, 0:D][:, None, :].to_broadcast([P, nb, D])
                lb = lnab[:, D:2 * D][:, None, :].to_broadcast([P, nb, D])
                yf3 = yf[:, :nb, :]
                nc.vector.tensor_mul(out=yf3, in0=yf3, in1=lg)
                nc.vector.tensor_add(out=yf3, in0=yf3, in1=lb)
            n0 = t0 * P
            dst = d_y[n0:n0 + nb * P, :].rearrange("(t p) d -> p t d", p=P)
            nc.sync.dma_start(dst, yf[:, :nb, :])

        for gg in range(NG):
            tg0 = gg * G
            # msg layout: [P, q, tile-in-group, c, D] -- q outermost so each
            # chunk's gather writes one contiguous [P, G*Cq, D] section
            msg = msgp.tile([P, NCHK * G * Cq * D], bf, tag="msg", name="msg")
            # split each chunk's gather in two on separate swdge queues so the
            # Q7 descriptor-gen of one half overlaps the other half's drain
            nrow = G * Cq                   # msg rows per (chunk, group)
            hrow = (nrow + 1) // 2
            for q in range(NCHK):
                icol = (q * NG + gg) * (IPG // 16)
                for h, (r0, r1) in enumerate(((0, hrow), (hrow, nrow))):
                    nh = (r1 - r0) * P
                    sec = msg[:, (q * nrow + r0) * D:(q * nrow + r1) * D]
                    nc.gpsimd.dma_gather(
                        out_ap=sec.rearrange("p (s d) -> p s d", d=D),
                        in_ap=d_gq[q][:],
                        idxs_ap=idxs[:, icol + r0 * 8:icol + r0 * 8 + nh // 16],
                        num_idxs=nh,
                        num_idxs_reg=nh,
                        elem_size=D,
                        single_packet=False,
                        queue_num=2 * q + h,
                    )
            for tl in range(G):
                t = tg0 + tl
                j = t % B
                if j == 0:
                    nb = min(B, T - t)
                    g4 = t // B
                    bt = (apsp.tile([P, B * D], f32, space="PSUM", tag="aps4",
                                    name="aps4"),
                          ratp.tile([P, B * D], f32, space="PSUM", tag="rps4",
                                    name="rps4"),
                          gamp.tile([P, B * D], f32, space="PSUM", tag="gps4",
                                    name="gps4"),
                          eltp.tile([P, B, D], f32, tag="rate4", name="rate4"))
                    # group bias matmuls open the psum banks (start=True zeroes
                    # the WHOLE 2KB bank, so it must come before any slice
                    # accumulation): agg += 2*cnt (x) b_fc ; gamma += b_rob
                    nc.tensor.matmul(out=bt[0][:, :nb * D],
                                     lhsT=cnt4[:, g4 * P:(g4 + 1) * P],
                                     rhs=bfc4[:, :nb * D],
                                     start=True, stop=False)
                    nc.tensor.matmul(out=bt[2][:, :nb * D],
                                     lhsT=onesr[:], rhs=brobt[:, :nb * D],
                                     start=True, stop=False)
                aps4, rps4, gps4, rate4 = bt
                last_of_group = (j == B - 1 or t == T - 1)
                sel = selp.tile([P, SELW], bf, tag="sel", name="sel")
                rb = rowsr[:, t * NCHK * Cq:(t + 1) * NCHK * Cq][:, :, None] \
                    .to_broadcast([P, NCHK * Cq, P])
                nc.vector.tensor_tensor(
                    out=sel.rearrange("p (c m) -> p c m", c=NCHK * Cq), in0=rb,
                    in1=iota.rearrange("p (c m) -> p c m", c=NCHK * Cq),
                    op=mybir.AluOpType.is_equal)
                # agg slice: self term (host-prescaled cnt*x) + neighbor sums
                nc.tensor.matmul(out=aps4[:, j * D:(j + 1) * D],
                                 lhsT=xls[:, t * P:(t + 1) * P], rhs=wfc[:],
                                 start=False, stop=False)
                for q in range(NCHK):
                    for c in range(Cq):
                        cc = q * Cq + c
                        moff = ((q * G + tl) * Cq + c) * D
                        nc.tensor.matmul(
                            out=aps4[:, j * D:(j + 1) * D],
                            lhsT=sel[:, cc * P:(cc + 1) * P],
                            rhs=msg[:, moff:moff + D],
                            start=False,
                            stop=(last_of_group and q == NCHK - 1
                                  and c == Cq - 1))
                # rate / gamma GEMMs for this tile
                nc.tensor.matmul(out=rps4[:, j * D:(j + 1) * D],
                                 lhsT=xloc[:, t * P:(t + 1) * P], rhs=wrt[:],
                                 start=True, stop=True)
                nc.tensor.matmul(out=gps4[:, j * D:(j + 1) * D],
                                 lhsT=xloc[:, t * P:(t + 1) * P], rhs=wrb[:],
                                 start=False, stop=last_of_group)
                if last_of_group:
                    nb = j + 1
                    # softplus = ln(exp(z) + 1) on the ACT engine, per group
                    spt = eltp.tile([P, B, D], f32, tag="spt", name="spt")
                    r4v = rps4[:, :nb * D].rearrange("p (b d) -> p b d", d=D)
                    nc.scalar.activation(out=spt[:, :nb, :], in_=r4v,
                                         func=AF.Exp)
                    nc.scalar.activation(out=rate4[:, :nb, :],
                                         in_=spt[:, :nb, :],
                                         func=AF.Ln, bias=1.0)
                    eltwise(bt, t - j, nb)

    nc.compile()
    return nc


def run(inputs, cfg: Cfg, core_ids=None):
    in_maps = prep(**inputs, cfg=cfg)
    nc = build(cfg)
    res = run_bass_kernel_spmd(nc, in_maps, core_ids=core_ids or list(range(cfg.NC)))
    ys = [res.results[r]["y"][:cfg.NLOC] for r in range(cfg.NC)]
    return np.concatenate(ys, axis=0)


def kernel(**inputs):
    cfg = Cfg(N=100_000, E=800_000, NC=8)
    return run(inputs, cfg)
